# revision 25
# baseline (speedup 1.0000x reference)
"""Trainium2 Bass kernel for nn_External_attention_44976897524182.

Math (folded):
  reference:
    y      = conv1_w @ x + conv1_b                    (1x1x1 conv = channel GEMM)
    logits = lin0_w @ y
    sm     = softmax(logits, axis=n)
    attn   = sm / (1e-9 + sum_k sm)
    z      = bn(conv2_w @ (lin1_w @ attn))
    out    = relu(z + x)
  All channel mixes are linear, so fold on host:
    A  = lin0_w @ conv1_w            (64 x 128)
    shift = bn_beta - bn_mean * bn_scale
    xs = x + shift                   (fold the BN shift into the residual)
    ab2 = lin0_w @ conv1_b - A @ shift   (bias correction for xs)
    B  = (bn_scale * conv2_w) @ lin1_w   (128 x 64)
  Then with e = exp(A@xs + ab2), S[k] = sum_n e[k,n] (global over n),
    denom[n] = sum_k e[k,n]/S[k]
    out[c,n] = relu( (B @ (e/S))[c,n] / denom[n] + xs[c,n] )
  (the 1e-9 is negligible: denom >= 2.5e-4 on this data)

Sharding: 8 cores = 2 batches x 4 n-slices of 32768. The only cross-core
dependency is S[k] (64 floats): AllGather over replica groups [[0-3],[4-7]]
plus a local 4-way sum.

Device: everything fp16 (inputs rounded on host; all matmuls 1 cyc/row,
half the HBM traffic; fp32 would be 4 cyc/row). invS is pre-scaled by 2^18
to keep B^T*invS in fp16 normal range -- the scale cancels between zz and
denom. Phase 1 keeps e resident in SBUF in a "paired" (128, 16384) fp16
layout: partitions 0-63 hold even 512-tiles, 64-127 odd tiles; xs stays
resident too (64KB/partition), so x is read from HBM exactly once.

Phase 2, per 128-position chunk, ONE psum bank receives two matmuls in one
accumulation group: [zz | denom] (128n x 129) = e_chunk^T @ [B^T*invS|invS]
plus xs^T (128n x 128) at offset 129 via a matmul against the identity
(on-device transpose; the xT matmul starts the bank group so it can issue
before invS is ready). The per-position r = 1/denom is then a per-partition
scalar: out_chunk = relu(zz*r + xsT) via one scalar_tensor_tensor (split
DVE/Pool since stt has no 2x mode) + one batched fp16 relu (4x on DVE).
Output is written n-major in a partition-major DRAM layout (128, 256, 128)
so every DMA descriptor moves >=2KB contiguous; host untangles it.
"""

import numpy as np

_B, _C = 2, 128
_D, _H, _W = 32, 64, 64
_N = _D * _H * _W          # 131072
_NCORES = 8
_SLICES = 4
_NSH = _N // _SLICES       # 32768 per core
_K = 64
_T = 512                   # phase-1 tile width
_BN_EPS = 1e-5
_ISCALE = 262144.0         # 2^18: keeps invS-scaled weights fp16-normal

_STT_DVE = 112             # (unused in the legal split below)
_STT_MOD = 2               # chunks with g%_STT_MOD==_STT_MOD-1 normalize via
                           # ACT copy-scale + Pool fp16 add (Pool cannot read
                           # PSUM and stt allows only one PSUM operand, so the
                           # bulk runs as DVE stt and the rest via this pair)
_PS2_TAGS = 6              # phase-2 psum banks in rotation
_PD_OWN = False            # denom pass on its own bank (vs sharing ps1 pp)
_CPB = 2                   # phase-2 chunks packed per psum bank
_TRF = 0                   # phase-1 pairs whose 8 chunks are transposed
                           # on-device (PE + batched copy); remaining chunks'
                           # xst slabs are host-uploaded instead (DMA/PE
                           # load balance, full fp16 precision either way)
_STT_CUT = 256             # chunks >= cut are all-DVE so Pool's queue drains
                           # early and the next rep's collective (dispatched
                           # from the Pool sequencer) isn't head-of-line
                           # blocked behind this rep's Pool stts

_nc_cache = None
last_results = None        # BassKernelResults of the most recent run


def _build(nsh=None, reps=1):
    global _nc_cache
    if nsh is None:
        nsh = _NSH
    full = nsh == _NSH and reps == 1
    if full and _nc_cache is not None:
        return _nc_cache
    NSH = nsh

    from contextlib import ExitStack
    import concourse.bass as bass  # noqa: F401
    import concourse.bacc as bacc
    import concourse.tile as tile
    import concourse.mybir as mybir

    f32 = mybir.dt.float32
    f16 = mybir.dt.float16

    nc = bacc.Bacc(
        trn_type="TRN2",
        target_bir_lowering=False,
        debug=False,
        num_devices=_NCORES,
    )
    x_d = nc.dram_tensor("x", [_C, NSH], f16, kind="ExternalInput").ap()
    at_d = nc.dram_tensor("a_t", [_C, _K], f16, kind="ExternalInput").ap()
    bt_d = nc.dram_tensor("b_t", [_K, _C], f32, kind="ExternalInput").ap()
    ab_d = nc.dram_tensor("ab2", [_C, 1], f32, kind="ExternalInput").ap()
    xst_d = nc.dram_tensor("xst", [_C, NSH // _C, _C], f16,
                           kind="ExternalInput").ap()
    eye_d = nc.dram_tensor("eye", [_C, _C], f16, kind="ExternalInput").ap()
    out_d = nc.dram_tensor("out", [_C, NSH // _C, _C], f16,
                           kind="ExternalOutput").ap()

    with tile.TileContext(nc) as tc, ExitStack() as ctx:
        consts = ctx.enter_context(tc.tile_pool(name="consts", bufs=1))
        big = ctx.enter_context(tc.tile_pool(name="big", bufs=2))
        xpool = ctx.enter_context(tc.tile_pool(name="xpool", bufs=2))
        stp = ctx.enter_context(tc.tile_pool(name="stp", bufs=3))
        sm2 = ctx.enter_context(tc.tile_pool(name="sm2", bufs=2))
        rp = ctx.enter_context(tc.tile_pool(name="rp", bufs=8))
        ps1 = ctx.enter_context(tc.tile_pool(name="ps1", bufs=2, space="PSUM"))
        ps2 = ctx.enter_context(tc.tile_pool(name="ps2", bufs=1, space="PSUM"))
        pst = ctx.enter_context(tc.tile_pool(name="pst", bufs=1, space="PSUM"))
        dram = ctx.enter_context(tc.tile_pool(name="dram", bufs=1, space="DRAM"))

        A_T = consts.tile([_C, _K], f16)
        nc.sync.dma_start(out=A_T, in_=at_d)
        B_T = consts.tile([_K, _C], f32)
        nc.sync.dma_start(out=B_T, in_=bt_d)
        ab2 = consts.tile([_C, 1], f32)
        nc.sync.dma_start(out=ab2, in_=ab_d)
        eye = consts.tile([_C, _C], f16)
        nc.sync.dma_start(out=eye, in_=eye_d)
        for _rep in range(reps):
            # double-buffered so rep i+1's phase 1 (and its collective) can
            # run entirely under rep i's phase 2
            xst_sb = big.tile([_C, NSH // _C, _C], f16, tag="xst")
            e_sb = big.tile([_C, NSH // 2], f16, tag="e")  # paired exp values
            _emit_body(nc, tc, mybir, NSH,
                       x_d, xst_d, eye, out_d, A_T, B_T, ab2, xst_sb,
                       e_sb, consts, sm2, stp, rp, xpool, ps1, ps2, pst, dram)

    nc.finalize()
    if full:
        _nc_cache = nc
    return nc


def _emit_body(nc, tc, mybir, NSH,
               x_d, xst_d, eye, out_d, A_T, B_T, ab2, xst_sb, e_sb,
               consts, sm2, stp, rp, xpool, ps1, ps2, pst, dram):
    import concourse.bass as _bass

    f32 = mybir.dt.float32
    f16 = mybir.dt.float16
    AF = mybir.ActivationFunctionType
    ALU = mybir.AluOpType

    NT = NSH // _T             # 512-tiles (64)
    NP = NT // 2               # pairs (32)
    NCH = NSH // _C            # 128-position chunks (256)

    # ---- phase 1: e = exp(A@xs + ab2), accumulate S ----
    # xs streams through small tiles (it is not needed after the matmul);
    # the n-major residual copy (xst) loads in 8 big slabs alongside.
    NJ = NSH // _C
    spart = consts.tile([_C, NP], f32, tag="spart")
    ncopy = [0]
    for p in range(NP):
        o = p * 2 * _T
        if p >= _TRF and (p - _TRF) % 4 == 0:
            s = (p - _TRF) // 4
            j0 = _TRF * 8 + s * 32
            nc.sync.dma_start(out=xst_sb[:, j0:j0 + 32, :],
                              in_=xst_d[:, j0:j0 + 32, :])
        xt = xpool.tile([_C, 2 * _T], f16, tag="xt")
        nc.sync.dma_start(out=xt, in_=x_d[:, o:o + 2 * _T])
        pp = ps1.tile([_C, _T], f32, tag="pp")
        nc.tensor.matmul(pp[0:_K, :], lhsT=A_T, rhs=xt[:, 0:_T],
                         start=True, stop=True)
        nc.tensor.matmul(pp[_K:_C, :], lhsT=A_T, rhs=xt[:, _T:2 * _T],
                         start=True, stop=True, tile_position=(0, _K))
        nc.scalar.activation(out=e_sb[:, p * _T:(p + 1) * _T], in_=pp,
                             func=AF.Exp, bias=ab2, scale=1.0,
                             accum_out=spart[:, p:p + 1])
        if p < _TRF:
            # transpose this pair's 8 chunks: 4 per psum bank, then one
            # batched copy to the resident n-major xst_sb
            for h in range(2):
                tb = pst.tile([_C, _T], f32, name="tb", tag=f"t{h}")
                for i in range(4):
                    nc.tensor.matmul(
                        tb[:, i * _C:(i + 1) * _C],
                        lhsT=xt[:, (4 * h + i) * _C:(4 * h + i + 1) * _C],
                        rhs=eye, start=(i == 0), stop=(i == 3),
                        skip_group_check=True)
                k = ncopy[0]
                ncopy[0] += 1
                dst = xst_sb[:, 8 * p + 4 * h:8 * p + 4 * h + 4, :]
                srcv = tb[:, :].rearrange("q (j c) -> q j c", j=4)
                if k % 2 == 0:
                    nc.scalar.copy(out=dst, in_=srcv)
                elif k % 4 == 1:
                    nc.gpsimd.tensor_copy(out=dst, in_=srcv)
                else:
                    nc.vector.tensor_copy(out=dst, in_=srcv)

    # ---- S: reduce partials, fold halves, AllGather + local sum ----
    sred = consts.tile([_C, 1], f32, tag="sred")
    nc.vector.tensor_reduce(out=sred, in_=spart,
                            axis=mybir.AxisListType.X, op=ALU.add)

    cc_in = dram.tile([_C, 1], f32, tag="cc_in")
    cc_out = dram.tile([_SLICES * _C, 1], f32, tag="cc_out")
    nc.sync.dma_start(out=cc_in, in_=sred)
    nc.gpsimd.collective_compute(
        "AllGather", ALU.bypass,
        replica_groups=[[0, 1, 2, 3], [4, 5, 6, 7]],
        ins=[cc_in.opt()], outs=[cc_out.opt()])
    sg8 = consts.tile([_K, 2 * _SLICES], f32, tag="sg8")
    nc.sync.dma_start(out=sg8,
                      in_=cc_out.rearrange("(g h k) o -> k (g h o)",
                                           g=_SLICES, h=2))
    sg = consts.tile([_K, 1], f32, tag="sg")
    nc.vector.tensor_reduce(out=sg, in_=sg8,
                            axis=mybir.AxisListType.X, op=ALU.add)
    invs = consts.tile([_K, 1], f32, tag="invs")
    nc.vector.reciprocal(out=invs, in_=sg)
    invs2 = consts.tile([_K, 1], f32, tag="invs2")
    nc.vector.tensor_scalar_mul(out=invs2, in0=invs, scalar1=_ISCALE)

    # rhs_aug = [B^T * invS' | invS'] fp16, duplicated into both halves
    rhs_aug = sm2.tile([_C, _C + 1], f16, tag="rhs_aug")
    nc.vector.tensor_scalar_mul(out=rhs_aug[0:_K, 0:_C], in0=B_T,
                                scalar1=invs2)
    nc.vector.tensor_copy(out=rhs_aug[0:_K, _C:_C + 1], in_=invs2)
    nc.sync.dma_start(out=rhs_aug[_K:_C, :], in_=rhs_aug[0:_K, :])

    # ---- phase 2a: all 256 denominators in one psum bank ----
    # denom[pos, g] = e_chunk_g^T @ invS'  (256 single-column matmuls, one
    # accumulation group; rhs_aug col 128 holds invS' in fp16). Then 8 block
    # reciprocals produce rr (128, 256) fp32 in SBUF, taking the reciprocal
    # entirely off the phase-2 steady-state dependency chain.
    def chunk_geom(g):
        t = g // 4
        return (t % 2) * _K, (t // 2) * _T + (g % 4) * _C

    pd = ps1.tile([_C, _T], f32, name="pd", tag="pp")
    for g in range(NCH):
        half, cs = chunk_geom(g)
        nc.tensor.matmul(
            pd[:, g:g + 1],
            lhsT=e_sb[half:half + _K, cs:cs + _C],
            rhs=rhs_aug[half:half + _K, _C:_C + 1],
            start=(g == 0), stop=(g == NCH - 1), skip_group_check=True)
    rr = sm2.tile([_C, NCH], f16, tag="rr")
    RB = NCH // 8
    with nc.allow_low_precision(reason="r=1/denom feeds an fp16 multiply; "
                                "fp16 r costs ~5e-4 rel on a term that is "
                                "~6% of the output"):
        for blk in range(8):
            nc.vector.reciprocal(out=rr[:, blk * RB:(blk + 1) * RB],
                                 in_=pd[:, blk * RB:(blk + 1) * RB])

    # ---- phase 2b: attention + residual, one chunk per psum bank ----
    # Bank layout [zz(128)|xT(128)] -- a 2-matmul accumulation group per
    # chunk that opens (xs-transpose, no invS dependency) and closes
    # (attention matmul) immediately, so the per-chunk scalar_tensor_tensor
    # (split DVE/Pool) depends only on its own chunk. Six single-bank tiles
    # rotate independently -> deep pipeline. Then a batched fp16 relu
    # (4x mode on DVE) and one >=2KB-contiguous DMA per 8 chunks.
    stage = None
    stt_acc = 0
    _pb_box = [None]
    pending = []               # (q, stage) awaiting relu + out DMA; emitted
                               # 2 stages late so the relu's multi-input sem
                               # wait never parks at the DVE queue head
    def flush_stage():
        q, stg = pending.pop(0)
        nc.vector.tensor_scalar_max(out=stg, in0=stg, scalar1=0.0)
        nc.sync.dma_start(out=out_d[:, q * 8:(q + 1) * 8, :], in_=stg)
    for g in range(NCH):
        half, cs = chunk_geom(g)
        n0 = g * _C
        if g % 8 == 0:
            stage = stp.tile([_C, 8, _C], f16, tag="stage")
        j = g % _CPB
        if j == 0:
            pb = ps2.tile([_C, _T], f32, name="pb",
                          tag=f"pb{(g // _CPB) % _PS2_TAGS}")
            _pb_box[0] = pb
        else:
            pb = _pb_box[0]
        nc.tensor.matmul(
            pb[:, j * _C:(j + 1) * _C],
            lhsT=e_sb[half:half + _K, cs:cs + _C],
            rhs=rhs_aug[half:half + _K, 0:_C],
            start=(j == 0), stop=(j == _CPB - 1 or g == NCH - 1),
            skip_group_check=True)
        in1 = xst_sb[:, g, :]
        if g % _STT_MOD == _STT_MOD - 1:
            tmp = rp.tile([_C, _C], f16, tag="tmp")
            nc.scalar.activation(out=tmp, in_=pb[:, j * _C:(j + 1) * _C],
                                 func=AF.Copy, bias=0.0,
                                 scale=rr[:, g:g + 1])
            nc.gpsimd.tensor_tensor(out=stage[:, g % 8, :], in0=tmp,
                                    in1=in1, op=ALU.add)
        else:
            nc.vector.scalar_tensor_tensor(
                out=stage[:, g % 8, :], in0=pb[:, j * _C:(j + 1) * _C],
                scalar=rr[:, g:g + 1],
                in1=in1,
                op0=ALU.mult, op1=ALU.add)
        if g % 8 == 7:
            pending.append((g // 8, stage))
            if len(pending) >= 2:
                flush_stage()
    while pending:
        flush_stage()


def _host_fold(inputs):
    f64 = np.float64
    lin0 = np.asarray(inputs["lin0_w"], f64)
    conv1 = np.asarray(inputs["conv1_w"], f64)
    conv1b = np.asarray(inputs["conv1_b"], f64)
    conv2 = np.asarray(inputs["conv2_w"], f64)
    lin1 = np.asarray(inputs["lin1_w"], f64)
    gamma = np.asarray(inputs["bn_gamma"], f64)
    beta = np.asarray(inputs["bn_beta"], f64)
    mean = np.asarray(inputs["bn_mean"], f64)
    var = np.asarray(inputs["bn_var"], f64)

    A = lin0 @ conv1                                            # (64,128)
    scale = gamma / np.sqrt(var + _BN_EPS)
    shift = beta - mean * scale                                 # (128,)
    ab = lin0 @ conv1b - A @ shift                              # (64,)
    Bm = (scale[:, None] * conv2) @ lin1                        # (128,64)
    return (A.astype(np.float32), ab.astype(np.float32),
            shift.astype(np.float32), Bm.astype(np.float32))


def _shard_inputs(inputs):
    x = np.asarray(inputs["x"], dtype=np.float32)
    A, ab, shift, Bm = _host_fold(inputs)

    a_t = np.ascontiguousarray(A.T).astype(np.float16)          # (128, 64)
    b_t = np.ascontiguousarray(Bm.T)                            # (64, 128) f32
    ab2 = np.concatenate([ab, ab]).reshape(_C, 1).astype(np.float32)

    xf = x.reshape(_B, _C, _N) + shift[None, :, None]
    xf16 = xf.astype(np.float16)
    in_maps = []
    for g in range(_NCORES):
        b = g // _SLICES
        s = g % _SLICES
        xs = xf16[b, :, s * _NSH:(s + 1) * _NSH]          # (128c, 32768)
        # xst[p, j, c] = xs[c, j*128 + p]
        xst = np.ascontiguousarray(
            xs.reshape(_C, _NSH // _C, _C).transpose(2, 1, 0))
        in_maps.append({
            "x": np.ascontiguousarray(xs),
            "xst": xst,
            "a_t": a_t,
            "b_t": b_t,
            "ab2": ab2,
            "eye": np.eye(_C, dtype=np.float16),
        })
    return in_maps


def kernel(**inputs):
    global last_results
    import time
    from concourse.bass_utils import run_bass_kernel_spmd

    in_maps = _shard_inputs(inputs)
    nc = _build()
    last_err = None
    for attempt in range(3):
        try:
            last_results = run_bass_kernel_spmd(
                nc, in_maps, core_ids=list(range(_NCORES)))
            break
        except Exception as e:  # transient axon worker hiccups: retry
            last_err = e
            if attempt == 2:
                raise
            time.sleep(20.0 * (attempt + 1))

    full = np.empty((_B, _C, _N), np.float32)
    for g in range(_NCORES):
        b = g // _SLICES
        s = g % _SLICES
        o = last_results.results[g]["out"]          # (128, 256, 128) f16
        o = o.transpose(1, 0, 2).reshape(_NSH, _C).astype(np.float32)
        full[b, :, s * _NSH:(s + 1) * _NSH] = o.T
    return full.reshape(_B, _C, _D, _H, _W)


# revision 28
# speedup vs baseline: 2.3491x; 2.3491x over previous
"""Trainium2 Bass kernel for nn_External_attention_44976897524182.

Math (folded on host, fp64):
  reference: y = conv1(x); attn = softmax(lin0 y, axis=n); attn /= sum_k attn
             out = relu(bn(conv2(lin1 attn)) + x)
  folded:    A = lin0@conv1 (64x128); shift = bn_beta - bn_mean*bn_scale
             xs = x + shift; ab2 = lin0@conv1_b - A@shift
             B = (bn_scale*conv2)@lin1 (128x64)
  with e = exp(A@xs + ab2), S[k] = sum_n e[k,n] (global over n):
             denom[n] = sum_k e[k,n]/S[k]
             out[c,n] = relu((B@(e/S))[c,n]/denom[n] + xs[c,n])
  (the reference's 1e-9 in the k-normalization is negligible: denom>=2.5e-4)

Sharding: 8 cores = 2 batches x 4 n-slices of 32768. The only cross-core
dependency is S[k]: AllGather (cheaper than AllReduce by its 1.875x cost
factor) over replica groups [[0-3],[4-7]] + local 8-way sum.

Device (all fp16: matmuls run 1 cyc/row vs 4 for fp32, HBM traffic halves;
max rel err ~7e-4 vs the 2e-2 gate):
- Phase 1 streams xs (c-major) through small tiles into A@xs matmuls, with
  exp+accumulate on the activation engine writing a resident paired-layout
  e (128, 16384): partitions 0-63 hold even 512-tiles, 64-127 odd ones.
  The n-major residual copy xst (host-prepared, partition-major layout
  (128, 256, 128) so every DMA descriptor moves >=2KB contiguous) loads in
  8 big slabs alongside. xst and e are double-buffered (pool bufs=2) so the
  next rep's phase 1 + collective overlap this rep's phase 2.
- After the AllGather: one 256-matmul accumulation group computes ALL
  chunk denominators into a single psum bank (rhs = invS' column, invS
  pre-scaled by 2^18 to stay fp16-normal -- the scale cancels between zz
  and denom), and 8 block reciprocals produce rr (128, 256) fp32 in SBUF,
  keeping reciprocals off the phase-2 steady-state chain.
- Phase 2: per 128-position chunk, zz = e_chunk^T @ (B^T invS') lands
  n-major in psum (2 chunks per bank, one start/stop group). Per-position
  normalize+residual: alternating chunks use one DVE scalar_tensor_tensor
  (zz*r + xst, the single allowed PSUM operand) or an ACT copy-with-scale
  plus Pool fp16 add (Pool cannot read PSUM). A batched fp16 relu (DVE 4x
  mode) + one >=2KB-contiguous DMA per 8 chunks, both emitted two stages
  late so their multi-input sem waits never park at a queue head.
Output is (128, 256, 128) fp16 partition-major; the host untangles,
transposes and casts.
"""

import numpy as np

_B, _C = 2, 128
_D, _H, _W = 32, 64, 64
_N = _D * _H * _W          # 131072
_NCORES = 8
_SLICES = 4
_NSH = _N // _SLICES       # 32768 per core
_K = 64
_T = 512                   # phase-1 tile width
_BN_EPS = 1e-5
_ISCALE = 262144.0         # 2^18: keeps invS-scaled weights fp16-normal

_STT_DVE = 112             # (unused in the legal split below)
_STT_MOD = 2               # chunks with g%_STT_MOD==_STT_MOD-1 normalize via
                           # ACT copy-scale + Pool fp16 add (Pool cannot read
                           # PSUM and stt allows only one PSUM operand, so the
                           # bulk runs as DVE stt and the rest via this pair)
_PS2_TAGS = 6              # phase-2 psum banks in rotation
_PD_OWN = False            # denom pass on its own bank (vs sharing ps1 pp)
_CPB = 2                   # phase-2 chunks packed per psum bank
_TRF = 0                   # phase-1 pairs whose 8 chunks are transposed
                           # on-device (PE + batched copy); remaining chunks'
                           # xst slabs are host-uploaded instead (DMA/PE
                           # load balance, full fp16 precision either way)
_STT_CUT = 256             # chunks >= cut are all-DVE so Pool's queue drains
                           # early and the next rep's collective (dispatched
                           # from the Pool sequencer) isn't head-of-line
                           # blocked behind this rep's Pool stts

_nc_cache = None
last_results = None        # BassKernelResults of the most recent run


def _build(nsh=None, reps=1):
    global _nc_cache
    if nsh is None:
        nsh = _NSH
    full = nsh == _NSH and reps == 1
    if full and _nc_cache is not None:
        return _nc_cache
    NSH = nsh

    from contextlib import ExitStack
    import concourse.bass as bass  # noqa: F401
    import concourse.bacc as bacc
    import concourse.tile as tile
    import concourse.mybir as mybir

    f32 = mybir.dt.float32
    f16 = mybir.dt.float16

    nc = bacc.Bacc(
        trn_type="TRN2",
        target_bir_lowering=False,
        debug=False,
        num_devices=_NCORES,
    )
    x_d = nc.dram_tensor("x", [_C, NSH], f16, kind="ExternalInput").ap()
    at_d = nc.dram_tensor("a_t", [_C, _K], f16, kind="ExternalInput").ap()
    bt_d = nc.dram_tensor("b_t", [_K, _C], f32, kind="ExternalInput").ap()
    ab_d = nc.dram_tensor("ab2", [_C, 1], f32, kind="ExternalInput").ap()
    xst_d = nc.dram_tensor("xst", [_C, NSH // _C, _C], f16,
                           kind="ExternalInput").ap()
    eye_d = nc.dram_tensor("eye", [_C, _C], f16, kind="ExternalInput").ap()
    out_d = nc.dram_tensor("out", [_C, NSH // _C, _C], f16,
                           kind="ExternalOutput").ap()

    with tile.TileContext(nc) as tc, ExitStack() as ctx:
        consts = ctx.enter_context(tc.tile_pool(name="consts", bufs=1))
        big = ctx.enter_context(tc.tile_pool(name="big", bufs=2))
        xpool = ctx.enter_context(tc.tile_pool(name="xpool", bufs=2))
        stp = ctx.enter_context(tc.tile_pool(name="stp", bufs=3))
        sm2 = ctx.enter_context(tc.tile_pool(name="sm2", bufs=2))
        rp = ctx.enter_context(tc.tile_pool(name="rp", bufs=4))
        ps1 = ctx.enter_context(tc.tile_pool(name="ps1", bufs=2, space="PSUM"))
        ps2 = ctx.enter_context(tc.tile_pool(name="ps2", bufs=1, space="PSUM"))
        pst = ctx.enter_context(tc.tile_pool(name="pst", bufs=1, space="PSUM"))
        dram = ctx.enter_context(tc.tile_pool(name="dram", bufs=1, space="DRAM"))

        A_T = consts.tile([_C, _K], f16)
        nc.sync.dma_start(out=A_T, in_=at_d)
        B_T = consts.tile([_K, _C], f32)
        nc.sync.dma_start(out=B_T, in_=bt_d)
        ab2 = consts.tile([_C, 1], f32)
        nc.sync.dma_start(out=ab2, in_=ab_d)
        eye = consts.tile([_C, _C], f16)
        nc.sync.dma_start(out=eye, in_=eye_d)
        for _rep in range(reps):
            # double-buffered so rep i+1's phase 1 (and its collective) can
            # run entirely under rep i's phase 2
            xst_sb = big.tile([_C, NSH // _C, _C], f16, tag="xst")
            e_sb = big.tile([_C, NSH // 2], f16, tag="e")  # paired exp values
            _emit_body(nc, tc, mybir, NSH,
                       x_d, xst_d, eye, out_d, A_T, B_T, ab2, xst_sb,
                       e_sb, consts, sm2, stp, rp, xpool, ps1, ps2, pst, dram)

    nc.finalize()
    if full:
        _nc_cache = nc
    return nc


def _emit_body(nc, tc, mybir, NSH,
               x_d, xst_d, eye, out_d, A_T, B_T, ab2, xst_sb, e_sb,
               consts, sm2, stp, rp, xpool, ps1, ps2, pst, dram):
    import concourse.bass as _bass

    f32 = mybir.dt.float32
    f16 = mybir.dt.float16
    AF = mybir.ActivationFunctionType
    ALU = mybir.AluOpType

    NT = NSH // _T             # 512-tiles (64)
    NP = NT // 2               # pairs (32)
    NCH = NSH // _C            # 128-position chunks (256)

    # ---- phase 1: e = exp(A@xs + ab2), accumulate S ----
    # xs streams through small tiles (it is not needed after the matmul);
    # the n-major residual copy (xst) loads in 8 big slabs alongside.
    NJ = NSH // _C
    spart = consts.tile([_C, NP], f32, tag="spart")
    ncopy = [0]
    for p in range(NP):
        o = p * 2 * _T
        if p >= _TRF and (p - _TRF) % 4 == 0:
            s = (p - _TRF) // 4
            j0 = _TRF * 8 + s * 32
            nc.sync.dma_start(out=xst_sb[:, j0:j0 + 32, :],
                              in_=xst_d[:, j0:j0 + 32, :])
        xt = xpool.tile([_C, 2 * _T], f16, tag="xt")
        nc.sync.dma_start(out=xt, in_=x_d[:, o:o + 2 * _T])
        pp = ps1.tile([_C, _T], f32, tag="pp")
        nc.tensor.matmul(pp[0:_K, :], lhsT=A_T, rhs=xt[:, 0:_T],
                         start=True, stop=True)
        nc.tensor.matmul(pp[_K:_C, :], lhsT=A_T, rhs=xt[:, _T:2 * _T],
                         start=True, stop=True, tile_position=(0, _K))
        nc.scalar.activation(out=e_sb[:, p * _T:(p + 1) * _T], in_=pp,
                             func=AF.Exp, bias=ab2, scale=1.0,
                             accum_out=spart[:, p:p + 1])
        if p < _TRF:
            # transpose this pair's 8 chunks: 4 per psum bank, then one
            # batched copy to the resident n-major xst_sb
            for h in range(2):
                tb = pst.tile([_C, _T], f32, name="tb", tag=f"t{h}")
                for i in range(4):
                    nc.tensor.matmul(
                        tb[:, i * _C:(i + 1) * _C],
                        lhsT=xt[:, (4 * h + i) * _C:(4 * h + i + 1) * _C],
                        rhs=eye, start=(i == 0), stop=(i == 3),
                        skip_group_check=True)
                k = ncopy[0]
                ncopy[0] += 1
                dst = xst_sb[:, 8 * p + 4 * h:8 * p + 4 * h + 4, :]
                srcv = tb[:, :].rearrange("q (j c) -> q j c", j=4)
                if k % 2 == 0:
                    nc.scalar.copy(out=dst, in_=srcv)
                elif k % 4 == 1:
                    nc.gpsimd.tensor_copy(out=dst, in_=srcv)
                else:
                    nc.vector.tensor_copy(out=dst, in_=srcv)

    # ---- S: reduce partials, fold halves, AllGather + local sum ----
    sred = consts.tile([_C, 1], f32, tag="sred")
    nc.vector.tensor_reduce(out=sred, in_=spart,
                            axis=mybir.AxisListType.X, op=ALU.add)

    cc_in = dram.tile([_C, 1], f32, tag="cc_in")
    cc_out = dram.tile([_SLICES * _C, 1], f32, tag="cc_out")
    nc.sync.dma_start(out=cc_in, in_=sred)
    nc.gpsimd.collective_compute(
        "AllGather", ALU.bypass,
        replica_groups=[[0, 1, 2, 3], [4, 5, 6, 7]],
        ins=[cc_in.opt()], outs=[cc_out.opt()])
    sg8 = consts.tile([_K, 2 * _SLICES], f32, tag="sg8")
    nc.sync.dma_start(out=sg8,
                      in_=cc_out.rearrange("(g h k) o -> k (g h o)",
                                           g=_SLICES, h=2))
    sg = consts.tile([_K, 1], f32, tag="sg")
    nc.vector.tensor_reduce(out=sg, in_=sg8,
                            axis=mybir.AxisListType.X, op=ALU.add)
    invs = consts.tile([_K, 1], f32, tag="invs")
    nc.vector.reciprocal(out=invs, in_=sg)
    invs2 = consts.tile([_K, 1], f32, tag="invs2")
    nc.vector.tensor_scalar_mul(out=invs2, in0=invs, scalar1=_ISCALE)

    # rhs_aug = [B^T * invS' | invS'] fp16, duplicated into both halves
    rhs_aug = sm2.tile([_C, _C + 1], f16, tag="rhs_aug")
    nc.vector.tensor_scalar_mul(out=rhs_aug[0:_K, 0:_C], in0=B_T,
                                scalar1=invs2)
    nc.vector.tensor_copy(out=rhs_aug[0:_K, _C:_C + 1], in_=invs2)
    nc.sync.dma_start(out=rhs_aug[_K:_C, :], in_=rhs_aug[0:_K, :])

    # ---- phase 2a: all 256 denominators in one psum bank ----
    # denom[pos, g] = e_chunk_g^T @ invS'  (256 single-column matmuls, one
    # accumulation group; rhs_aug col 128 holds invS' in fp16). Then 8 block
    # reciprocals produce rr (128, 256) fp32 in SBUF, taking the reciprocal
    # entirely off the phase-2 steady-state dependency chain.
    def chunk_geom(g):
        t = g // 4
        return (t % 2) * _K, (t // 2) * _T + (g % 4) * _C

    pd = ps1.tile([_C, _T], f32, name="pd", tag="pp")
    for g in range(NCH):
        half, cs = chunk_geom(g)
        nc.tensor.matmul(
            pd[:, g:g + 1],
            lhsT=e_sb[half:half + _K, cs:cs + _C],
            rhs=rhs_aug[half:half + _K, _C:_C + 1],
            start=(g == 0), stop=(g == NCH - 1), skip_group_check=True)
    rr = sm2.tile([_C, NCH], f32, tag="rr")
    RB = NCH // 8
    for blk in range(8):
        nc.vector.reciprocal(out=rr[:, blk * RB:(blk + 1) * RB],
                             in_=pd[:, blk * RB:(blk + 1) * RB])

    # ---- phase 2b: attention + residual, one chunk per psum bank ----
    # Bank layout [zz(128)|xT(128)] -- a 2-matmul accumulation group per
    # chunk that opens (xs-transpose, no invS dependency) and closes
    # (attention matmul) immediately, so the per-chunk scalar_tensor_tensor
    # (split DVE/Pool) depends only on its own chunk. Six single-bank tiles
    # rotate independently -> deep pipeline. Then a batched fp16 relu
    # (4x mode on DVE) and one >=2KB-contiguous DMA per 8 chunks.
    stage = None
    stt_acc = 0
    _pb_box = [None]
    for g in range(NCH):
        half, cs = chunk_geom(g)
        n0 = g * _C
        if g % 8 == 0:
            stage = stp.tile([_C, 8, _C], f16, tag="stage")
        j = g % _CPB
        if j == 0:
            pb = ps2.tile([_C, _T], f32, name="pb",
                          tag=f"pb{(g // _CPB) % _PS2_TAGS}")
            _pb_box[0] = pb
        else:
            pb = _pb_box[0]
        nc.tensor.matmul(
            pb[:, j * _C:(j + 1) * _C],
            lhsT=e_sb[half:half + _K, cs:cs + _C],
            rhs=rhs_aug[half:half + _K, 0:_C],
            start=(j == 0), stop=(j == _CPB - 1 or g == NCH - 1),
            skip_group_check=True)
        in1 = xst_sb[:, g, :]
        if g % _STT_MOD == _STT_MOD - 1:
            tmp = rp.tile([_C, _C], f16, tag="tmp")
            nc.scalar.activation(out=tmp, in_=pb[:, j * _C:(j + 1) * _C],
                                 func=AF.Copy, bias=0.0,
                                 scale=rr[:, g:g + 1])
            nc.gpsimd.tensor_tensor(out=stage[:, g % 8, :], in0=tmp,
                                    in1=in1, op=ALU.add)
        else:
            nc.vector.scalar_tensor_tensor(
                out=stage[:, g % 8, :], in0=pb[:, j * _C:(j + 1) * _C],
                scalar=rr[:, g:g + 1],
                in1=in1,
                op0=ALU.mult, op1=ALU.add)
        if g % 8 == 7:
            q = g // 8
            nc.vector.tensor_scalar_max(out=stage, in0=stage, scalar1=0.0)
            nc.sync.dma_start(out=out_d[:, q * 8:(q + 1) * 8, :], in_=stage)


def _host_fold(inputs):
    f64 = np.float64
    lin0 = np.asarray(inputs["lin0_w"], f64)
    conv1 = np.asarray(inputs["conv1_w"], f64)
    conv1b = np.asarray(inputs["conv1_b"], f64)
    conv2 = np.asarray(inputs["conv2_w"], f64)
    lin1 = np.asarray(inputs["lin1_w"], f64)
    gamma = np.asarray(inputs["bn_gamma"], f64)
    beta = np.asarray(inputs["bn_beta"], f64)
    mean = np.asarray(inputs["bn_mean"], f64)
    var = np.asarray(inputs["bn_var"], f64)

    A = lin0 @ conv1                                            # (64,128)
    scale = gamma / np.sqrt(var + _BN_EPS)
    shift = beta - mean * scale                                 # (128,)
    ab = lin0 @ conv1b - A @ shift                              # (64,)
    Bm = (scale[:, None] * conv2) @ lin1                        # (128,64)
    return (A.astype(np.float32), ab.astype(np.float32),
            shift.astype(np.float32), Bm.astype(np.float32))


def _shard_inputs(inputs):
    x = np.asarray(inputs["x"], dtype=np.float32)
    A, ab, shift, Bm = _host_fold(inputs)

    a_t = np.ascontiguousarray(A.T).astype(np.float16)          # (128, 64)
    b_t = np.ascontiguousarray(Bm.T)                            # (64, 128) f32
    ab2 = np.concatenate([ab, ab]).reshape(_C, 1).astype(np.float32)

    xf = x.reshape(_B, _C, _N) + shift[None, :, None]
    xf16 = xf.astype(np.float16)
    in_maps = []
    for g in range(_NCORES):
        b = g // _SLICES
        s = g % _SLICES
        xs = xf16[b, :, s * _NSH:(s + 1) * _NSH]          # (128c, 32768)
        # xst[p, j, c] = xs[c, j*128 + p]
        xst = np.ascontiguousarray(
            xs.reshape(_C, _NSH // _C, _C).transpose(2, 1, 0))
        in_maps.append({
            "x": np.ascontiguousarray(xs),
            "xst": xst,
            "a_t": a_t,
            "b_t": b_t,
            "ab2": ab2,
            "eye": np.eye(_C, dtype=np.float16),
        })
    return in_maps


def kernel(**inputs):
    global last_results
    import time
    from concourse.bass_utils import run_bass_kernel_spmd

    in_maps = _shard_inputs(inputs)
    nc = _build()
    last_err = None
    for attempt in range(3):
        try:
            last_results = run_bass_kernel_spmd(
                nc, in_maps, core_ids=list(range(_NCORES)))
            break
        except Exception as e:  # transient axon worker hiccups: retry
            last_err = e
            if attempt == 2:
                raise
            time.sleep(20.0 * (attempt + 1))

    full = np.empty((_B, _C, _N), np.float32)
    for g in range(_NCORES):
        b = g // _SLICES
        s = g % _SLICES
        o = last_results.results[g]["out"]          # (128, 256, 128) f16
        o = o.transpose(1, 0, 2).reshape(_NSH, _C).astype(np.float32)
        full[b, :, s * _NSH:(s + 1) * _NSH] = o.T
    return full.reshape(_B, _C, _D, _H, _W)


# revision 35
# speedup vs baseline: 11.0704x; 4.7127x over previous
"""Trainium2 Bass kernel for nn_External_attention_44976897524182.

Math (folded on host, fp64):
  reference: y = conv1(x); attn = softmax(lin0 y, axis=n); attn /= sum_k attn
             out = relu(bn(conv2(lin1 attn)) + x)
  folded:    A = lin0@conv1 (64x128); shift = bn_beta - bn_mean*bn_scale
             xs = x + shift; ab2 = lin0@conv1_b - A@shift
             B = (bn_scale*conv2)@lin1 (128x64)
  with e = exp(A@xs + ab2), S[k] = sum_n e[k,n] (global over n):
             denom[n] = sum_k e[k,n]/S[k]
             out[c,n] = relu((B@(e/S))[c,n]/denom[n] + xs[c,n])
  (the reference's 1e-9 in the k-normalization is negligible: denom>=2.5e-4)

Sharding: 8 cores = 2 batches x 4 n-slices of 32768. The only cross-core
dependency is S[k]: AllGather (cheaper than AllReduce by its 1.875x cost
factor) over replica groups [[0-3],[4-7]] + local 8-way sum.

Device (all fp16: matmuls run 1 cyc/row vs 4 for fp32, HBM traffic halves;
max rel err ~7e-4 vs the 2e-2 gate):
- Phase 1 streams xs (c-major) through small tiles into A@xs matmuls, with
  exp+accumulate on the activation engine writing a resident paired-layout
  e (128, 16384): partitions 0-63 hold even 512-tiles, 64-127 odd ones.
  The n-major residual copy xst (host-prepared, partition-major layout
  (128, 256, 128) so every DMA descriptor moves >=2KB contiguous) loads in
  8 big slabs alongside. xst and e are double-buffered (pool bufs=2) so the
  next rep's phase 1 + collective overlap this rep's phase 2.
- After the AllGather: one 256-matmul accumulation group computes ALL
  chunk denominators into a single psum bank (rhs = invS' column, invS
  pre-scaled by 2^18 to stay fp16-normal -- the scale cancels between zz
  and denom), and 8 block reciprocals produce rr (128, 256) fp32 in SBUF,
  keeping reciprocals off the phase-2 steady-state chain.
- Phase 2: per 128-position chunk, zz = e_chunk^T @ (B^T invS') lands
  n-major in psum (2 chunks per bank, one start/stop group). Per-position
  normalize+residual: alternating chunks use one DVE scalar_tensor_tensor
  (zz*r + xst, the single allowed PSUM operand) or an ACT copy-with-scale
  plus Pool fp16 add (Pool cannot read PSUM). A batched fp16 relu (DVE 4x
  mode) + one >=2KB-contiguous DMA per 8 chunks, both emitted two stages
  late so their multi-input sem waits never park at a queue head.
Output is (128, 256, 128) fp16 partition-major; the host untangles,
transposes and casts.
"""

import numpy as np

_B, _C = 2, 128
_D, _H, _W = 32, 64, 64
_N = _D * _H * _W          # 131072
_NCORES = 8
_SLICES = 4
_NSH = _N // _SLICES       # 32768 per core
_K = 64
_T = 512                   # phase-1 tile width
_BN_EPS = 1e-5
_ISCALE = 262144.0         # 2^18: keeps invS-scaled weights fp16-normal

_STT_DVE = 112             # (unused in the legal split below)
_STT_PAT = "mod2"          # which chunks use the ACT+Pool normalize path.
                           # "early208"/"early232" sim ~10% faster
                           # (Pool queue drains before the next rep's
                           # collective dispatches) but mesh-desynced the
                           # PJRT timing path 0/2; "mod2" passed it 2/2


def _use_actpool(g):
    if _STT_PAT == "mod2":
        return g % 2 == 1
    if _STT_PAT == "2of5":
        return g % 5 in (2, 4)
    if _STT_PAT == "early":
        return g < 180 and g % 2 == 1
    if _STT_PAT == "early160":
        return g < 160 and g % 2 == 1
    if _STT_PAT == "early208":
        return g < 208 and g % 2 == 1
    if _STT_PAT == "early232":
        return g < 232 and g % 2 == 1
    return False


_STT_MOD = 2               # chunks with g%_STT_MOD==_STT_MOD-1 normalize via
                           # ACT copy-scale + Pool fp16 add (Pool cannot read
                           # PSUM and stt allows only one PSUM operand, so the
                           # bulk runs as DVE stt and the rest via this pair)
_PS2_TAGS = 6              # phase-2 psum banks in rotation
_PD_OWN = False            # denom pass on its own bank (vs sharing ps1 pp)
_CPB = 2                   # phase-2 chunks packed per psum bank
_TRF = 0                   # phase-1 pairs whose 8 chunks are transposed
                           # on-device (PE + batched copy); remaining chunks'
                           # xst slabs are host-uploaded instead (DMA/PE
                           # load balance, full fp16 precision either way)
_STT_CUT = 256             # chunks >= cut are all-DVE so Pool's queue drains
                           # early and the next rep's collective (dispatched
                           # from the Pool sequencer) isn't head-of-line
                           # blocked behind this rep's Pool stts

_nc_cache = None
last_results = None        # BassKernelResults of the most recent run


def _build(nsh=None, reps=1):
    global _nc_cache
    if nsh is None:
        nsh = _NSH
    full = nsh == _NSH and reps == 1
    if full and _nc_cache is not None:
        return _nc_cache
    NSH = nsh

    from contextlib import ExitStack
    import concourse.bass as bass  # noqa: F401
    import concourse.bacc as bacc
    import concourse.tile as tile
    import concourse.mybir as mybir

    f32 = mybir.dt.float32
    f16 = mybir.dt.float16

    nc = bacc.Bacc(
        trn_type="TRN2",
        target_bir_lowering=False,
        debug=False,
        num_devices=_NCORES,
    )
    x_d = nc.dram_tensor("x", [_C, NSH], f16, kind="ExternalInput").ap()
    at_d = nc.dram_tensor("a_t", [_C, _K], f16, kind="ExternalInput").ap()
    bt_d = nc.dram_tensor("b_t", [_K, _C], f32, kind="ExternalInput").ap()
    ab_d = nc.dram_tensor("ab2", [_C, 1], f32, kind="ExternalInput").ap()
    xst_d = nc.dram_tensor("xst", [_C, NSH // _C, _C], f16,
                           kind="ExternalInput").ap()
    eye_d = nc.dram_tensor("eye", [_C, _C], f16, kind="ExternalInput").ap()
    out_d = nc.dram_tensor("out", [_C, NSH // _C, _C], f16,
                           kind="ExternalOutput").ap()

    with tile.TileContext(nc) as tc, ExitStack() as ctx:
        consts = ctx.enter_context(tc.tile_pool(name="consts", bufs=1))
        big = ctx.enter_context(tc.tile_pool(name="big", bufs=2))
        xpool = ctx.enter_context(tc.tile_pool(name="xpool", bufs=2))
        stp = ctx.enter_context(tc.tile_pool(name="stp", bufs=3))
        sm2 = ctx.enter_context(tc.tile_pool(name="sm2", bufs=2))
        rp = ctx.enter_context(tc.tile_pool(name="rp", bufs=4))
        ps1 = ctx.enter_context(tc.tile_pool(name="ps1", bufs=2, space="PSUM"))
        ps2 = ctx.enter_context(tc.tile_pool(name="ps2", bufs=1, space="PSUM"))
        pst = ctx.enter_context(tc.tile_pool(name="pst", bufs=1, space="PSUM"))
        dram = ctx.enter_context(tc.tile_pool(name="dram", bufs=1, space="DRAM"))

        A_T = consts.tile([_C, _K], f16)
        nc.sync.dma_start(out=A_T, in_=at_d)
        B_T = consts.tile([_K, _C], f32)
        nc.sync.dma_start(out=B_T, in_=bt_d)
        ab2 = consts.tile([_C, 1], f32)
        nc.sync.dma_start(out=ab2, in_=ab_d)
        eye = consts.tile([_C, _C], f16)
        nc.sync.dma_start(out=eye, in_=eye_d)
        for _rep in range(reps):
            # double-buffered so rep i+1's phase 1 (and its collective) can
            # run entirely under rep i's phase 2
            xst_sb = big.tile([_C, NSH // _C, _C], f16, tag="xst")
            e_sb = big.tile([_C, NSH // 2], f16, tag="e")  # paired exp values
            _emit_body(nc, tc, mybir, NSH,
                       x_d, xst_d, eye, out_d, A_T, B_T, ab2, xst_sb,
                       e_sb, consts, sm2, stp, rp, xpool, ps1, ps2, pst, dram)

    nc.finalize()
    if full:
        _nc_cache = nc
    return nc


def _emit_body(nc, tc, mybir, NSH,
               x_d, xst_d, eye, out_d, A_T, B_T, ab2, xst_sb, e_sb,
               consts, sm2, stp, rp, xpool, ps1, ps2, pst, dram):
    import concourse.bass as _bass

    f32 = mybir.dt.float32
    f16 = mybir.dt.float16
    AF = mybir.ActivationFunctionType
    ALU = mybir.AluOpType

    NT = NSH // _T             # 512-tiles (64)
    NP = NT // 2               # pairs (32)
    NCH = NSH // _C            # 128-position chunks (256)

    # ---- phase 1: e = exp(A@xs + ab2), accumulate S ----
    # xs streams through small tiles (it is not needed after the matmul);
    # the n-major residual copy (xst) loads in 8 big slabs alongside.
    NJ = NSH // _C
    spart = consts.tile([_C, NP], f32, tag="spart")
    ncopy = [0]
    for p in range(NP):
        o = p * 2 * _T
        if p >= _TRF and (p - _TRF) % 4 == 0:
            s = (p - _TRF) // 4
            j0 = _TRF * 8 + s * 32
            nc.sync.dma_start(out=xst_sb[:, j0:j0 + 32, :],
                              in_=xst_d[:, j0:j0 + 32, :])
        xt = xpool.tile([_C, 2 * _T], f16, tag="xt")
        nc.sync.dma_start(out=xt, in_=x_d[:, o:o + 2 * _T])
        pp = ps1.tile([_C, _T], f32, tag="pp")
        nc.tensor.matmul(pp[0:_K, :], lhsT=A_T, rhs=xt[:, 0:_T],
                         start=True, stop=True)
        nc.tensor.matmul(pp[_K:_C, :], lhsT=A_T, rhs=xt[:, _T:2 * _T],
                         start=True, stop=True, tile_position=(0, _K))
        nc.scalar.activation(out=e_sb[:, p * _T:(p + 1) * _T], in_=pp,
                             func=AF.Exp, bias=ab2, scale=1.0,
                             accum_out=spart[:, p:p + 1])
        if p < _TRF:
            # transpose this pair's 8 chunks: 4 per psum bank, then one
            # batched copy to the resident n-major xst_sb
            for h in range(2):
                tb = pst.tile([_C, _T], f32, name="tb", tag=f"t{h}")
                for i in range(4):
                    nc.tensor.matmul(
                        tb[:, i * _C:(i + 1) * _C],
                        lhsT=xt[:, (4 * h + i) * _C:(4 * h + i + 1) * _C],
                        rhs=eye, start=(i == 0), stop=(i == 3),
                        skip_group_check=True)
                k = ncopy[0]
                ncopy[0] += 1
                dst = xst_sb[:, 8 * p + 4 * h:8 * p + 4 * h + 4, :]
                srcv = tb[:, :].rearrange("q (j c) -> q j c", j=4)
                if k % 2 == 0:
                    nc.scalar.copy(out=dst, in_=srcv)
                elif k % 4 == 1:
                    nc.gpsimd.tensor_copy(out=dst, in_=srcv)
                else:
                    nc.vector.tensor_copy(out=dst, in_=srcv)

    # ---- S: reduce partials, fold halves, AllGather + local sum ----
    sred = consts.tile([_C, 1], f32, tag="sred")
    nc.vector.tensor_reduce(out=sred, in_=spart,
                            axis=mybir.AxisListType.X, op=ALU.add)

    cc_in = dram.tile([_C, 1], f32, tag="cc_in")
    cc_out = dram.tile([_SLICES * _C, 1], f32, tag="cc_out")
    nc.sync.dma_start(out=cc_in, in_=sred)
    nc.gpsimd.collective_compute(
        "AllGather", ALU.bypass,
        replica_groups=[[0, 1, 2, 3], [4, 5, 6, 7]],
        ins=[cc_in.opt()], outs=[cc_out.opt()])
    sg8 = consts.tile([_K, 2 * _SLICES], f32, tag="sg8")
    nc.sync.dma_start(out=sg8,
                      in_=cc_out.rearrange("(g h k) o -> k (g h o)",
                                           g=_SLICES, h=2))
    sg = consts.tile([_K, 1], f32, tag="sg")
    nc.vector.tensor_reduce(out=sg, in_=sg8,
                            axis=mybir.AxisListType.X, op=ALU.add)
    invs = consts.tile([_K, 1], f32, tag="invs")
    nc.vector.reciprocal(out=invs, in_=sg)
    invs2 = consts.tile([_K, 1], f32, tag="invs2")
    nc.vector.tensor_scalar_mul(out=invs2, in0=invs, scalar1=_ISCALE)

    # rhs_aug = [B^T * invS' | invS'] fp16, duplicated into both halves
    rhs_aug = sm2.tile([_C, _C + 1], f16, tag="rhs_aug")
    nc.vector.tensor_scalar_mul(out=rhs_aug[0:_K, 0:_C], in0=B_T,
                                scalar1=invs2)
    nc.vector.tensor_copy(out=rhs_aug[0:_K, _C:_C + 1], in_=invs2)
    nc.sync.dma_start(out=rhs_aug[_K:_C, :], in_=rhs_aug[0:_K, :])

    # ---- phase 2a: all 256 denominators in one psum bank ----
    # denom[pos, g] = e_chunk_g^T @ invS'  (256 single-column matmuls, one
    # accumulation group; rhs_aug col 128 holds invS' in fp16). Then 8 block
    # reciprocals produce rr (128, 256) fp32 in SBUF, taking the reciprocal
    # entirely off the phase-2 steady-state dependency chain.
    def chunk_geom(g):
        t = g // 4
        return (t % 2) * _K, (t // 2) * _T + (g % 4) * _C

    pd = ps1.tile([_C, _T], f32, name="pd", tag="pp")
    for g in range(NCH):
        half, cs = chunk_geom(g)
        nc.tensor.matmul(
            pd[:, g:g + 1],
            lhsT=e_sb[half:half + _K, cs:cs + _C],
            rhs=rhs_aug[half:half + _K, _C:_C + 1],
            start=(g == 0), stop=(g == NCH - 1), skip_group_check=True)
    rr = sm2.tile([_C, NCH], f32, tag="rr")
    RB = NCH // 8
    for blk in range(8):
        nc.vector.reciprocal(out=rr[:, blk * RB:(blk + 1) * RB],
                             in_=pd[:, blk * RB:(blk + 1) * RB])

    # ---- phase 2b: attention + residual, one chunk per psum bank ----
    # Bank layout [zz(128)|xT(128)] -- a 2-matmul accumulation group per
    # chunk that opens (xs-transpose, no invS dependency) and closes
    # (attention matmul) immediately, so the per-chunk scalar_tensor_tensor
    # (split DVE/Pool) depends only on its own chunk. Six single-bank tiles
    # rotate independently -> deep pipeline. Then a batched fp16 relu
    # (4x mode on DVE) and one >=2KB-contiguous DMA per 8 chunks.
    stage = None
    stt_acc = 0
    _pb_box = [None]
    for g in range(NCH):
        half, cs = chunk_geom(g)
        n0 = g * _C
        if g % 8 == 0:
            stage = stp.tile([_C, 8, _C], f16, tag="stage")
        j = g % _CPB
        if j == 0:
            pb = ps2.tile([_C, _T], f32, name="pb",
                          tag=f"pb{(g // _CPB) % _PS2_TAGS}")
            _pb_box[0] = pb
        else:
            pb = _pb_box[0]
        nc.tensor.matmul(
            pb[:, j * _C:(j + 1) * _C],
            lhsT=e_sb[half:half + _K, cs:cs + _C],
            rhs=rhs_aug[half:half + _K, 0:_C],
            start=(j == 0), stop=(j == _CPB - 1 or g == NCH - 1),
            skip_group_check=True)
        in1 = xst_sb[:, g, :]
        if _use_actpool(g):
            tmp = rp.tile([_C, _C], f16, tag="tmp")
            nc.scalar.activation(out=tmp, in_=pb[:, j * _C:(j + 1) * _C],
                                 func=AF.Copy, bias=0.0,
                                 scale=rr[:, g:g + 1])
            nc.gpsimd.tensor_tensor(out=stage[:, g % 8, :], in0=tmp,
                                    in1=in1, op=ALU.add)
        else:
            nc.vector.scalar_tensor_tensor(
                out=stage[:, g % 8, :], in0=pb[:, j * _C:(j + 1) * _C],
                scalar=rr[:, g:g + 1],
                in1=in1,
                op0=ALU.mult, op1=ALU.add)
        if g % 8 == 7:
            q = g // 8
            nc.vector.tensor_scalar_max(out=stage, in0=stage, scalar1=0.0)
            nc.sync.dma_start(out=out_d[:, q * 8:(q + 1) * 8, :], in_=stage)


def _host_fold(inputs):
    f64 = np.float64
    lin0 = np.asarray(inputs["lin0_w"], f64)
    conv1 = np.asarray(inputs["conv1_w"], f64)
    conv1b = np.asarray(inputs["conv1_b"], f64)
    conv2 = np.asarray(inputs["conv2_w"], f64)
    lin1 = np.asarray(inputs["lin1_w"], f64)
    gamma = np.asarray(inputs["bn_gamma"], f64)
    beta = np.asarray(inputs["bn_beta"], f64)
    mean = np.asarray(inputs["bn_mean"], f64)
    var = np.asarray(inputs["bn_var"], f64)

    A = lin0 @ conv1                                            # (64,128)
    scale = gamma / np.sqrt(var + _BN_EPS)
    shift = beta - mean * scale                                 # (128,)
    ab = lin0 @ conv1b - A @ shift                              # (64,)
    Bm = (scale[:, None] * conv2) @ lin1                        # (128,64)
    return (A.astype(np.float32), ab.astype(np.float32),
            shift.astype(np.float32), Bm.astype(np.float32))


def _shard_inputs(inputs):
    x = np.asarray(inputs["x"], dtype=np.float32)
    A, ab, shift, Bm = _host_fold(inputs)

    a_t = np.ascontiguousarray(A.T).astype(np.float16)          # (128, 64)
    b_t = np.ascontiguousarray(Bm.T)                            # (64, 128) f32
    ab2 = np.concatenate([ab, ab]).reshape(_C, 1).astype(np.float32)

    xf = x.reshape(_B, _C, _N) + shift[None, :, None]
    xf16 = xf.astype(np.float16)
    in_maps = []
    for g in range(_NCORES):
        b = g // _SLICES
        s = g % _SLICES
        xs = xf16[b, :, s * _NSH:(s + 1) * _NSH]          # (128c, 32768)
        # xst[p, j, c] = xs[c, j*128 + p]
        xst = np.ascontiguousarray(
            xs.reshape(_C, _NSH // _C, _C).transpose(2, 1, 0))
        in_maps.append({
            "x": np.ascontiguousarray(xs),
            "xst": xst,
            "a_t": a_t,
            "b_t": b_t,
            "ab2": ab2,
            "eye": np.eye(_C, dtype=np.float16),
        })
    return in_maps


def kernel(**inputs):
    global last_results
    import time
    from concourse.bass_utils import run_bass_kernel_spmd

    in_maps = _shard_inputs(inputs)
    nc = _build()
    last_err = None
    for attempt in range(3):
        try:
            last_results = run_bass_kernel_spmd(
                nc, in_maps, core_ids=list(range(_NCORES)))
            break
        except Exception as e:  # transient axon worker hiccups: retry
            last_err = e
            if attempt == 2:
                raise
            time.sleep(20.0 * (attempt + 1))

    full = np.empty((_B, _C, _N), np.float32)
    for g in range(_NCORES):
        b = g // _SLICES
        s = g % _SLICES
        o = last_results.results[g]["out"]          # (128, 256, 128) f16
        o = o.transpose(1, 0, 2).reshape(_NSH, _C).astype(np.float32)
        full[b, :, s * _NSH:(s + 1) * _NSH] = o.T
    return full.reshape(_B, _C, _D, _H, _W)
